# revision 21
# baseline (speedup 1.0000x reference)
"""Trainium2 Bass kernel for the quirky-softmax attention head.

Math (reference):
    Q = query @ Wq + bq ; K = key @ Wk + bk ; V = value @ Wv + bv     [S, D]
    e = exp(Q K^T / D)                                               [S, S]
    weights[i, j] = e[i, j] / rs[j],  rs[j] = sum_k e[j, k]          (column-indexed norm)
    out = weights @ V                                                [S, D]

Strategy (8 NeuronCores, sequence-parallel, single SPMD NEFF):
  * Host pre-transposes query/key/value to [D, S] and hands core c its
    512-column slice, plus the full (reshaped) weights.
  * Core c computes QT_c = Wq^T-form projection of its queries (kept in SBUF),
    and its own 512-row blocks of K^T and V; each block is AllGather'd
    (2 MB/rank, bf16) as soon as it is produced -- AG(K^T) hides behind the
    V/Q projections, AG(V) behind the whole scores phase.
  * Scores are computed directly transposed, ET[j, m] = exp((Q_c K^T)^T / D),
    so the output matmul needs no on-chip transpose. Row sums come from a
    ones-vector matmul accumulated across all key chunks; the per-core [512]
    sums are AllGather'd (2 KB/rank) into the full rs[4096].
  * out_c = (ET * (1/rs)[j]) ^T-contraction @ V, accumulated over all 32 key
    chunks in 8 PSUM banks.
  * All matmuls run in bf16 with fp32 PSUM accumulation (full PE rate and
    half the HBM traffic vs fp32; measured rel err vs the fp32 reference is
    ~2.2e-3). PSUM pool = 8 banks shared by tag across all phases.
"""

import numpy as np
import ml_dtypes

BF = ml_dtypes.bfloat16

S = 4096
D = 1024
NCORES = 8
P = 128
SB = S // NCORES          # 512 queries (and keys) owned per core
DC = D // P               # 8 contraction chunks over D
JCL = SB // P             # 4 local 128-key chunks per core block
RCH = S // P              # 32 global 128-key chunks
NH = D // 512             # 2 halves of the output feature dim

_CACHE = {}


def _build_nc(sim_mode=False):
    import concourse.tile as tile
    from concourse import bacc, mybir

    F32 = mybir.dt.float32
    BF16 = mybir.dt.bfloat16
    FP8 = mybir.dt.float8e4
    AF = mybir.ActivationFunctionType
    RG = [list(range(NCORES))]

    nc = bacc.Bacc("TRN2", target_bir_lowering=False, debug=False,
                   num_devices=NCORES)

    qt = nc.dram_tensor("qt", [P, DC * SB], BF16, kind="ExternalInput").ap()
    kt = nc.dram_tensor("kt", [P, DC * SB], BF16, kind="ExternalInput").ap()
    vt = nc.dram_tensor("vt", [P, DC * SB], BF16, kind="ExternalInput").ap()
    wq = nc.dram_tensor("wq", [P, DC * D], BF16, kind="ExternalInput").ap()
    wk = nc.dram_tensor("wk", [P, DC * D], BF16, kind="ExternalInput").ap()
    wv = nc.dram_tensor("wv", [P, DC * D], BF16, kind="ExternalInput").ap()
    bq = nc.dram_tensor("bq", [P, DC], F32, kind="ExternalInput").ap()
    bk = nc.dram_tensor("bk", [P, DC], F32, kind="ExternalInput").ap()
    bv = nc.dram_tensor("bv", [1, D], BF16, kind="ExternalInput").ap()
    onesc = nc.dram_tensor("onesc", [P, 1], BF16, kind="ExternalInput").ap()
    onesr = nc.dram_tensor("onesr", [1, P], BF16, kind="ExternalInput").ap()
    out = nc.dram_tensor("out", [SB, D], F32, kind="ExternalOutput").ap()

    BLK = DC * P * SB     # 524288 elements in one core's K^T (or V) block

    with tile.TileContext(nc) as tc:
        with (
            tc.tile_pool(name="dram", bufs=1, space="DRAM") as dram,
            tc.tile_pool(name="consts", bufs=1) as consts,
            tc.tile_pool(name="qtp", bufs=1) as qtp,
            tc.tile_pool(name="etp", bufs=1) as etp,
            tc.tile_pool(name="psum", bufs=8, space="PSUM") as psum,
        ):
            kt_ag_in = dram.tile([DC, P, SB], FP8)
            kt_ag_out = dram.tile([NCORES, DC, P, SB], FP8,
                                  addr_space="Local" if sim_mode else "Shared")
            v_ag_in = dram.tile([JCL, P, D], BF16)
            v_ag_out = dram.tile([NCORES, JCL, P, D], BF16,
                                 addr_space="Local" if sim_mode else "Shared")
            rs_in = dram.tile([1, SB], F32)
            rs_out = dram.tile([NCORES, SB], F32,
                               addr_space="Local" if sim_mode else "Shared")


            qt_sb = qtp.tile([P, DC * SB], FP8)       # QT_c resident, fp8
            et_sb = etp.tile([P, RCH * SB], BF16)     # ET resident    (8 MB)

            # ---------------- phase 1: projections + AllGather(KT, V) -------
            with (
                tc.tile_pool(name="inp", bufs=1) as inp,
                tc.tile_pool(name="wp", bufs=4) as wp,
                tc.tile_pool(name="pop", bufs=6) as pop,
            ):
                kt_in = inp.tile([P, DC * SB], BF16)
                vt_in = inp.tile([P, DC * SB], BF16)
                qt_in = inp.tile([P, DC * SB], BF16)

                # KT_c[dout, j'] = sum_d Wk[d, dout] keyT[d, j'] + bk[dout]
                kt_ps = [psum.tile([P, SB], F32, tag="ps", name=f"ktps{m}")
                         for m in range(DC)]
                for dc in range(DC):
                    nc.sync.dma_start(out=kt_in[:, dc * SB:(dc + 1) * SB],
                                      in_=kt[:, dc * SB:(dc + 1) * SB])
                    wk_t = wp.tile([P, D], BF16, tag="w", name=f"wk{dc}")
                    nc.sync.dma_start(out=wk_t[:], in_=wk[:, dc * D:(dc + 1) * D])
                    for mc in range(DC):
                        nc.tensor.matmul(
                            kt_ps[mc][:],
                            wk_t[:, mc * P:(mc + 1) * P],
                            kt_in[:, dc * SB:(dc + 1) * SB],
                            start=(dc == 0), stop=(dc == DC - 1))
                wv_t0 = wp.tile([P, D], BF16, tag="w", name="wv_pre")
                nc.sync.dma_start(out=vt_in[:, 0:SB], in_=vt[:, 0:SB])
                nc.sync.dma_start(out=wv_t0[:], in_=wv[:, 0:D])

                bq_sb = consts.tile([P, DC], F32)
                nc.sync.dma_start(out=bq_sb[:], in_=bq)
                bk_sb = consts.tile([P, DC], F32)
                nc.sync.dma_start(out=bk_sb[:], in_=bk)
                bv_sb = consts.tile([1, D], BF16)
                nc.sync.dma_start(out=bv_sb[:], in_=bv)
                ones_col = consts.tile([P, 1], BF16)
                nc.sync.dma_start(out=ones_col[:], in_=onesc)
                ones_row = consts.tile([1, P], BF16)
                nc.sync.dma_start(out=ones_row[:], in_=onesr)
                for mc in range(DC):
                    kt_o = pop.tile([P, SB], FP8, tag="po8", name=f"kto{mc}")
                    nc.scalar.activation(kt_o[:], kt_ps[mc][:], AF.Identity,
                                         bias=bk_sb[:, mc:mc + 1])
                    nc.sync.dma_start(out=kt_ag_in[mc], in_=kt_o[:])

                if sim_mode:
                    for r in range(NCORES):
                        nc.sync.dma_start(out=kt_ag_out[r, :, :, 0:64],
                                          in_=kt_ag_in[:, :, 0:64])
                else:
                    nc.gpsimd.collective_compute(
                        "AllGather", mybir.AluOpType.bypass, replica_groups=RG,
                        ins=[kt_ag_in.opt()], outs=[kt_ag_out.opt()])

                # V_c[j', n] = sum_d valueT[d, j'] Wv[d, n] + bv[n]
                v_ps = [psum.tile([P, 512], F32, tag="ps", name=f"vps{i}")
                        for i in range(JCL * NH)]
                for dc in range(DC):
                    if dc == 0:
                        wv_t = wv_t0
                    else:
                        nc.sync.dma_start(out=vt_in[:, dc * SB:(dc + 1) * SB],
                                          in_=vt[:, dc * SB:(dc + 1) * SB])
                        wv_t = wp.tile([P, D], BF16, tag="w", name=f"wv{dc}")
                        nc.sync.dma_start(out=wv_t[:], in_=wv[:, dc * D:(dc + 1) * D])
                    for jc in range(JCL):
                        for h in range(NH):
                            nc.tensor.matmul(
                                v_ps[jc * NH + h][:],
                                vt_in[:, dc * SB + jc * P: dc * SB + (jc + 1) * P],
                                wv_t[:, h * 512:(h + 1) * 512],
                                start=(dc == 0), stop=False)
                for jc in range(JCL):
                    for h in range(NH):
                        nc.tensor.matmul(
                            v_ps[jc * NH + h][:],
                            ones_row,
                            bv_sb[:, h * 512:(h + 1) * 512],
                            start=False, stop=True)
                        v_o = pop.tile([P, 512], BF16, tag="po", name=f"vo{jc}{h}")
                        nc.scalar.activation(v_o[:], v_ps[jc * NH + h][:], AF.Copy)
                        nc.sync.dma_start(out=v_ag_in[jc, :, h * 512:(h + 1) * 512],
                                          in_=v_o[:])

                if sim_mode:
                    for r in range(NCORES):
                        nc.sync.dma_start(out=v_ag_out[r, :, :, 0:128],
                                          in_=v_ag_in[:, :, 0:128])
                else:
                    nc.gpsimd.collective_compute(
                        "AllGather", mybir.AluOpType.bypass, replica_groups=RG,
                        ins=[v_ag_in.opt()], outs=[v_ag_out.opt()])

                # QT_c[dout, m] = sum_d Wq[d, dout] queryT[d, m] + bq[dout]
                wq_t0 = wp.tile([P, D], BF16, tag="w", name="wq_pre")
                nc.sync.dma_start(out=qt_in[:, 0:SB], in_=qt[:, 0:SB])
                nc.sync.dma_start(out=wq_t0[:], in_=wq[:, 0:D])
                q_ps = [psum.tile([P, SB], F32, tag="ps", name=f"qps{m}")
                        for m in range(DC)]
                for dc in range(DC):
                    if dc == 0:
                        wq_t = wq_t0
                    else:
                        nc.sync.dma_start(out=qt_in[:, dc * SB:(dc + 1) * SB],
                                          in_=qt[:, dc * SB:(dc + 1) * SB])
                        wq_t = wp.tile([P, D], BF16, tag="w", name=f"wq{dc}")
                        nc.sync.dma_start(out=wq_t[:], in_=wq[:, dc * D:(dc + 1) * D])
                    for mc in range(DC):
                        nc.tensor.matmul(
                            q_ps[mc][:],
                            wq_t[:, mc * P:(mc + 1) * P],
                            qt_in[:, dc * SB:(dc + 1) * SB],
                            start=(dc == 0), stop=(dc == DC - 1))
                for mc in range(DC):
                    nc.scalar.activation(qt_sb[:, mc * SB:(mc + 1) * SB],
                                         q_ps[mc][:], AF.Identity,
                                         bias=bq_sb[:, mc:mc + 1])

            # ---------------- phases 2+3: scores/exp/rowsums, then output ---
            with (
                tc.tile_pool(name="ktb", bufs=6) as ktbp,
                tc.tile_pool(name="vst", bufs=8) as vstp,
                tc.tile_pool(name="op", bufs=4) as op,
                tc.tile_pool(name="small", bufs=1) as sp,
            ):
                # scores^T per 128-key chunk: ET[j, m] = exp(scores[m, j]/D)
                rs_ps = psum.tile([1, SB], F32, tag="ps")
                for r in range(NCORES):
                    ktb = ktbp.tile([P, DC * SB], FP8, tag="ktb", name=f"ktb{r}")
                    for dc in range(DC):
                        nc.sync.dma_start(out=ktb[:, dc * SB:(dc + 1) * SB],
                                          in_=kt_ag_out[r, dc])
                    for jj in range(JCL):
                        jc = r * JCL + jj
                        s_ps = psum.tile([P, SB], F32, tag="ps", name=f"sps{jc}")
                        ktb3 = ktb.rearrange("p (dc j) -> p dc j", dc=DC)
                        qt3 = qt_sb.rearrange("p (dc m) -> p dc m", dc=DC)
                        for c2 in range(DC // 2):
                            nc.tensor.matmul(
                                s_ps[:],
                                ktb3[:, 2 * c2:2 * c2 + 2, jj * P:(jj + 1) * P],
                                qt3[:, 2 * c2:2 * c2 + 2, :],
                                start=(c2 == 0), stop=(c2 == DC // 2 - 1),
                                perf_mode=mybir.MatmulPerfMode.DoubleRow)
                        nc.scalar.activation(et_sb[:, jc * SB:(jc + 1) * SB],
                                             s_ps[:], AF.Exp, scale=1.0 / D)
                        nc.tensor.matmul(
                            rs_ps[:], ones_col,
                            et_sb[:, jc * SB:(jc + 1) * SB],
                            start=(jc == 0), stop=(jc == RCH - 1))

                # rs AllGather + reciprocal, partition-major for per-key scaling
                rs_sb = sp.tile([1, SB], F32)
                nc.vector.tensor_copy(rs_sb[:], rs_ps[:])
                nc.sync.dma_start(out=rs_in[:], in_=rs_sb[:])
                if sim_mode:
                    nc.sync.dma_start(out=rs_out[:, :],
                                      in_=rs_in.to_broadcast([NCORES, SB]))
                else:
                    nc.gpsimd.collective_compute(
                        "AllGather", mybir.AluOpType.bypass, replica_groups=RG,
                        ins=[rs_in.opt()], outs=[rs_out.opt()])
                rs32_sb = sp.tile([RCH, P], F32)
                nc.sync.dma_start(
                    out=rs32_sb[:],
                    in_=rs_out.rearrange("r m -> (r m)").rearrange(
                        "(jc p) -> jc p", p=P))
                rs_p_sb = sp.tile([P, RCH], F32)
                for q in range(P // 32):
                    nc.vector.transpose(rs_p_sb[q * 32:(q + 1) * 32, 0:32],
                                        rs32_sb[0:32, q * 32:(q + 1) * 32])
                recip_sb = sp.tile([P, RCH], F32)
                nc.vector.reciprocal(recip_sb[:], rs_p_sb[:])

                # out_c[m, n] = sum_j ET[j, m] * (1/rs[j]) * V[j, n]
                out_ps = [psum.tile([P, 512], F32, tag="ps", name=f"ops{i}")
                          for i in range(DC // 2 * NH)]
                for r in range(NCORES):
                    vv = v_ag_out[r]
                    for jj in range(JCL):
                        jc = r * JCL + jj
                        v_t = vstp.tile([P, D], BF16, tag="v", name=f"v{jc}")
                        nc.sync.dma_start(out=v_t[:], in_=vv[jj])
                        nc.vector.tensor_scalar_mul(
                            et_sb[:, jc * SB:(jc + 1) * SB],
                            et_sb[:, jc * SB:(jc + 1) * SB],
                            recip_sb[:, jc:jc + 1])
                        for mc in range(SB // P):
                            for h in range(NH):
                                nc.tensor.matmul(
                                    out_ps[mc * NH + h][:],
                                    et_sb[:, jc * SB + mc * P: jc * SB + (mc + 1) * P],
                                    v_t[:, h * 512:(h + 1) * 512],
                                    start=(jc == 0), stop=(jc == RCH - 1))
                for mc in range(SB // P):
                    for h in range(NH):
                        o_t = op.tile([P, 512], F32, tag="o", name=f"o{mc}{h}")
                        if (mc * NH + h) % 2 == 0:
                            nc.vector.tensor_copy(o_t[:], out_ps[mc * NH + h][:])
                        else:
                            nc.scalar.activation(o_t[:], out_ps[mc * NH + h][:],
                                                 AF.Copy)
                        nc.sync.dma_start(
                            out=out[mc * P:(mc + 1) * P, h * 512:(h + 1) * 512],
                            in_=o_t[:])

    nc.compile()
    return nc


def get_nc():
    if "nc" not in _CACHE:
        _CACHE["nc"] = _build_nc()
    return _CACHE["nc"]


def _chunked_cols(a):
    """[D, X] -> [128, (D//128) * X] bf16 with d-chunk-major columns."""
    x = a.shape[1]
    return np.ascontiguousarray(
        a.reshape(DC, P, x).transpose(1, 0, 2).reshape(P, DC * x).astype(BF))


def prepare_in_maps(inputs):
    query = np.asarray(inputs["query"], dtype=np.float32)
    key = np.asarray(inputs["key"], dtype=np.float32)
    value = np.asarray(inputs["value"], dtype=np.float32)
    qT, kT, vT = query.T, key.T, value.T
    wq = _chunked_cols(np.asarray(inputs["Wq"], dtype=np.float32))
    wk = _chunked_cols(np.asarray(inputs["Wk"], dtype=np.float32))
    wv = _chunked_cols(np.asarray(inputs["Wv"], dtype=np.float32))
    bq = np.ascontiguousarray(
        np.asarray(inputs["bq"], dtype=np.float32).reshape(DC, P).T)
    bk = np.ascontiguousarray(
        np.asarray(inputs["bk"], dtype=np.float32).reshape(DC, P).T)
    bv = np.ascontiguousarray(
        np.asarray(inputs["bv"], dtype=np.float32).reshape(1, D).astype(BF))
    in_maps = []
    for c in range(NCORES):
        sl = slice(c * SB, (c + 1) * SB)
        in_maps.append({
            "qt": _chunked_cols(np.ascontiguousarray(qT[:, sl])),
            "kt": _chunked_cols(np.ascontiguousarray(kT[:, sl])),
            "vt": _chunked_cols(np.ascontiguousarray(vT[:, sl])),
            "wq": wq, "wk": wk, "wv": wv,
            "bq": bq, "bk": bk, "bv": bv,
            "onesc": np.ones((P, 1), dtype=BF),
            "onesr": np.ones((1, P), dtype=BF),
        })
    return in_maps


def kernel(**inputs):
    from concourse.bass_utils import run_bass_kernel_spmd

    nc = get_nc()
    in_maps = prepare_in_maps(inputs)
    res = run_bass_kernel_spmd(nc, in_maps, core_ids=list(range(NCORES)))
    return np.ascontiguousarray(
        np.concatenate([r["out"] for r in res.results], axis=0))


if __name__ == "__main__":
    rng = np.random.default_rng(0)
    ins = {
        "query": rng.standard_normal((S, D), dtype=np.float32),
        "key": rng.standard_normal((S, D), dtype=np.float32),
        "value": rng.standard_normal((S, D), dtype=np.float32),
        "Wq": rng.standard_normal((D, D), dtype=np.float32) * 0.05,
        "bq": rng.standard_normal((D,), dtype=np.float32) * 0.05,
        "Wk": rng.standard_normal((D, D), dtype=np.float32) * 0.05,
        "bk": rng.standard_normal((D,), dtype=np.float32) * 0.05,
        "Wv": rng.standard_normal((D, D), dtype=np.float32) * 0.05,
        "bv": rng.standard_normal((D,), dtype=np.float32) * 0.05,
    }
    got = kernel(**ins)
    print("kernel output", got.shape, got.dtype)


# revision 22
# speedup vs baseline: 1.1642x; 1.1642x over previous
"""Trainium2 Bass kernel for the quirky-softmax attention head.

Math (reference):
    Q = query @ Wq + bq ; K = key @ Wk + bk ; V = value @ Wv + bv     [S, D]
    e = exp(Q K^T / D)                                               [S, S]
    weights[i, j] = e[i, j] / rs[j],  rs[j] = sum_k e[j, k]          (column-indexed norm)
    out = weights @ V                                                [S, D]

Strategy (8 NeuronCores, sequence-parallel, single SPMD NEFF):
  * Host pre-transposes query/key/value to [D, S] and hands core c its
    512-column slice, plus the full (reshaped) weights.
  * Core c computes QT_c = Wq^T-form projection of its queries (kept in SBUF),
    and its own 512-row blocks of K^T and V; each block is AllGather'd
    (2 MB/rank, bf16) as soon as it is produced -- AG(K^T) hides behind the
    V/Q projections, AG(V) behind the whole scores phase.
  * Scores are computed directly transposed, ET[j, m] = exp((Q_c K^T)^T / D),
    so the output matmul needs no on-chip transpose. Row sums come from a
    ones-vector matmul accumulated across all key chunks; the per-core [512]
    sums are AllGather'd (2 KB/rank) into the full rs[4096].
  * out_c = (ET * (1/rs)[j]) ^T-contraction @ V, accumulated over all 32 key
    chunks in 8 PSUM banks.
  * All matmuls run in bf16 with fp32 PSUM accumulation (full PE rate and
    half the HBM traffic vs fp32; measured rel err vs the fp32 reference is
    ~2.2e-3). PSUM pool = 8 banks shared by tag across all phases.
"""

import numpy as np
import ml_dtypes

BF = ml_dtypes.bfloat16

S = 4096
D = 1024
NCORES = 8
P = 128
SB = S // NCORES          # 512 queries (and keys) owned per core
DC = D // P               # 8 contraction chunks over D
JCL = SB // P             # 4 local 128-key chunks per core block
RCH = S // P              # 32 global 128-key chunks
NH = D // 512             # 2 halves of the output feature dim

_CACHE = {}


def _build_nc(sim_mode=False):
    import concourse.tile as tile
    from concourse import bacc, mybir

    F32 = mybir.dt.float32
    BF16 = mybir.dt.bfloat16
    FP8 = mybir.dt.float8e4
    AF = mybir.ActivationFunctionType
    RG = [list(range(NCORES))]

    nc = bacc.Bacc("TRN2", target_bir_lowering=False, debug=False,
                   num_devices=NCORES)

    qt = nc.dram_tensor("qt", [P, DC * SB], FP8, kind="ExternalInput").ap()
    kt = nc.dram_tensor("kt", [P, DC * SB], FP8, kind="ExternalInput").ap()
    vt = nc.dram_tensor("vt", [P, DC * SB], BF16, kind="ExternalInput").ap()
    wq = nc.dram_tensor("wq", [P, DC * D], FP8, kind="ExternalInput").ap()
    wk = nc.dram_tensor("wk", [P, DC * D], FP8, kind="ExternalInput").ap()
    wv = nc.dram_tensor("wv", [P, DC * D], BF16, kind="ExternalInput").ap()
    bq = nc.dram_tensor("bq", [P, DC], F32, kind="ExternalInput").ap()
    bk = nc.dram_tensor("bk", [P, DC], F32, kind="ExternalInput").ap()
    bv = nc.dram_tensor("bv", [1, D], BF16, kind="ExternalInput").ap()
    onesc = nc.dram_tensor("onesc", [P, 1], BF16, kind="ExternalInput").ap()
    onesr = nc.dram_tensor("onesr", [1, P], BF16, kind="ExternalInput").ap()
    out = nc.dram_tensor("out", [SB, D], F32, kind="ExternalOutput").ap()

    BLK = DC * P * SB     # 524288 elements in one core's K^T (or V) block

    with tile.TileContext(nc) as tc:
        with (
            tc.tile_pool(name="dram", bufs=1, space="DRAM") as dram,
            tc.tile_pool(name="consts", bufs=1) as consts,
            tc.tile_pool(name="qtp", bufs=1) as qtp,
            tc.tile_pool(name="etp", bufs=1) as etp,
            tc.tile_pool(name="psum", bufs=8, space="PSUM") as psum,
        ):
            kt_ag_in = dram.tile([DC, P, SB], FP8)
            kt_ag_out = dram.tile([NCORES, DC, P, SB], FP8,
                                  addr_space="Local" if sim_mode else "Shared")
            v_ag_in = dram.tile([JCL, P, D], BF16)
            v_ag_out = dram.tile([NCORES, JCL, P, D], BF16,
                                 addr_space="Local" if sim_mode else "Shared")
            rs_in = dram.tile([1, SB], F32)
            rs_out = dram.tile([NCORES, SB], F32,
                               addr_space="Local" if sim_mode else "Shared")


            qt_sb = qtp.tile([P, DC * SB], FP8)       # QT_c resident, fp8
            et_sb = etp.tile([P, RCH * SB], BF16)     # ET resident    (8 MB)

            # ---------------- phase 1: projections + AllGather(KT, V) -------
            with (
                tc.tile_pool(name="inp", bufs=1) as inp,
                tc.tile_pool(name="wp", bufs=4) as wp,
                tc.tile_pool(name="pop", bufs=6) as pop,
            ):
                kt_in = inp.tile([P, DC * SB], FP8)
                vt_in = inp.tile([P, DC * SB], BF16)
                qt_in = inp.tile([P, DC * SB], FP8)

                # KT_c[dout, j'] = sum_d Wk[d, dout] keyT[d, j'] + bk[dout]
                kt_ps = [psum.tile([P, SB], F32, tag="ps", name=f"ktps{m}")
                         for m in range(DC)]
                for c2 in range(DC // 2):
                    kt_w2 = wp.tile([P, 2, D], FP8, tag="w", name=f"wk{c2}")
                    for u in range(2):
                        dc = 2 * c2 + u
                        nc.sync.dma_start(out=kt_in[:, dc * SB:(dc + 1) * SB],
                                          in_=kt[:, dc * SB:(dc + 1) * SB])
                        nc.sync.dma_start(out=kt_w2[:, u], in_=wk[:, dc * D:(dc + 1) * D])
                    kt_in3 = kt_in.rearrange("p (dc j) -> p dc j", dc=DC)
                    for mc in range(DC):
                        nc.tensor.matmul(
                            kt_ps[mc][:],
                            kt_w2[:, :, mc * P:(mc + 1) * P],
                            kt_in3[:, 2 * c2:2 * c2 + 2, :],
                            start=(c2 == 0), stop=(c2 == DC // 2 - 1),
                            perf_mode=mybir.MatmulPerfMode.DoubleRow)
                wv_t0 = wp.tile([P, D], BF16, tag="w", name="wv_pre")
                nc.sync.dma_start(out=vt_in[:, 0:SB], in_=vt[:, 0:SB])
                nc.sync.dma_start(out=wv_t0[:], in_=wv[:, 0:D])

                bq_sb = consts.tile([P, DC], F32)
                nc.sync.dma_start(out=bq_sb[:], in_=bq)
                bk_sb = consts.tile([P, DC], F32)
                nc.sync.dma_start(out=bk_sb[:], in_=bk)
                bv_sb = consts.tile([1, D], BF16)
                nc.sync.dma_start(out=bv_sb[:], in_=bv)
                ones_col = consts.tile([P, 1], BF16)
                nc.sync.dma_start(out=ones_col[:], in_=onesc)
                ones_row = consts.tile([1, P], BF16)
                nc.sync.dma_start(out=ones_row[:], in_=onesr)
                for mc in range(DC):
                    kt_o = pop.tile([P, SB], FP8, tag="po8", name=f"kto{mc}")
                    nc.scalar.activation(kt_o[:], kt_ps[mc][:], AF.Identity,
                                         bias=bk_sb[:, mc:mc + 1])
                    nc.sync.dma_start(out=kt_ag_in[mc], in_=kt_o[:])

                if sim_mode:
                    for r in range(NCORES):
                        nc.sync.dma_start(out=kt_ag_out[r, :, :, 0:64],
                                          in_=kt_ag_in[:, :, 0:64])
                else:
                    nc.gpsimd.collective_compute(
                        "AllGather", mybir.AluOpType.bypass, replica_groups=RG,
                        ins=[kt_ag_in.opt()], outs=[kt_ag_out.opt()])

                # V_c[j', n] = sum_d valueT[d, j'] Wv[d, n] + bv[n]
                v_ps = [psum.tile([P, 512], F32, tag="ps", name=f"vps{i}")
                        for i in range(JCL * NH)]
                for dc in range(DC):
                    if dc == 0:
                        wv_t = wv_t0
                    else:
                        nc.sync.dma_start(out=vt_in[:, dc * SB:(dc + 1) * SB],
                                          in_=vt[:, dc * SB:(dc + 1) * SB])
                        wv_t = wp.tile([P, D], BF16, tag="w", name=f"wv{dc}")
                        nc.sync.dma_start(out=wv_t[:], in_=wv[:, dc * D:(dc + 1) * D])
                    for jc in range(JCL):
                        for h in range(NH):
                            nc.tensor.matmul(
                                v_ps[jc * NH + h][:],
                                vt_in[:, dc * SB + jc * P: dc * SB + (jc + 1) * P],
                                wv_t[:, h * 512:(h + 1) * 512],
                                start=(dc == 0), stop=False)
                for jc in range(JCL):
                    for h in range(NH):
                        nc.tensor.matmul(
                            v_ps[jc * NH + h][:],
                            ones_row,
                            bv_sb[:, h * 512:(h + 1) * 512],
                            start=False, stop=True)
                        v_o = pop.tile([P, 512], BF16, tag="po", name=f"vo{jc}{h}")
                        nc.scalar.activation(v_o[:], v_ps[jc * NH + h][:], AF.Copy)
                        nc.sync.dma_start(out=v_ag_in[jc, :, h * 512:(h + 1) * 512],
                                          in_=v_o[:])

                if sim_mode:
                    for r in range(NCORES):
                        nc.sync.dma_start(out=v_ag_out[r, :, :, 0:128],
                                          in_=v_ag_in[:, :, 0:128])
                else:
                    nc.gpsimd.collective_compute(
                        "AllGather", mybir.AluOpType.bypass, replica_groups=RG,
                        ins=[v_ag_in.opt()], outs=[v_ag_out.opt()])

                # QT_c[dout, m] = sum_d Wq[d, dout] queryT[d, m] + bq[dout]
                q_ps = [psum.tile([P, SB], F32, tag="ps", name=f"qps{m}")
                        for m in range(DC)]
                for c2 in range(DC // 2):
                    qt_w2 = wp.tile([P, 2, D], FP8, tag="w", name=f"wq{c2}")
                    for u in range(2):
                        dc = 2 * c2 + u
                        nc.sync.dma_start(out=qt_in[:, dc * SB:(dc + 1) * SB],
                                          in_=qt[:, dc * SB:(dc + 1) * SB])
                        nc.sync.dma_start(out=qt_w2[:, u], in_=wq[:, dc * D:(dc + 1) * D])
                    qt_in3 = qt_in.rearrange("p (dc m) -> p dc m", dc=DC)
                    for mc in range(DC):
                        nc.tensor.matmul(
                            q_ps[mc][:],
                            qt_w2[:, :, mc * P:(mc + 1) * P],
                            qt_in3[:, 2 * c2:2 * c2 + 2, :],
                            start=(c2 == 0), stop=(c2 == DC // 2 - 1),
                            perf_mode=mybir.MatmulPerfMode.DoubleRow)
                for mc in range(DC):
                    nc.scalar.activation(qt_sb[:, mc * SB:(mc + 1) * SB],
                                         q_ps[mc][:], AF.Identity,
                                         bias=bq_sb[:, mc:mc + 1])

            # ---------------- phases 2+3: scores/exp/rowsums, then output ---
            with (
                tc.tile_pool(name="ktb", bufs=6) as ktbp,
                tc.tile_pool(name="vst", bufs=8) as vstp,
                tc.tile_pool(name="op", bufs=4) as op,
                tc.tile_pool(name="small", bufs=1) as sp,
            ):
                # scores^T per 128-key chunk: ET[j, m] = exp(scores[m, j]/D)
                rs_ps = psum.tile([1, SB], F32, tag="ps")
                for r in range(NCORES):
                    ktb = ktbp.tile([P, DC * SB], FP8, tag="ktb", name=f"ktb{r}")
                    for dc in range(DC):
                        nc.sync.dma_start(out=ktb[:, dc * SB:(dc + 1) * SB],
                                          in_=kt_ag_out[r, dc])
                    for jj in range(JCL):
                        jc = r * JCL + jj
                        s_ps = psum.tile([P, SB], F32, tag="ps", name=f"sps{jc}")
                        ktb3 = ktb.rearrange("p (dc j) -> p dc j", dc=DC)
                        qt3 = qt_sb.rearrange("p (dc m) -> p dc m", dc=DC)
                        for c2 in range(DC // 2):
                            nc.tensor.matmul(
                                s_ps[:],
                                ktb3[:, 2 * c2:2 * c2 + 2, jj * P:(jj + 1) * P],
                                qt3[:, 2 * c2:2 * c2 + 2, :],
                                start=(c2 == 0), stop=(c2 == DC // 2 - 1),
                                perf_mode=mybir.MatmulPerfMode.DoubleRow)
                        nc.scalar.activation(et_sb[:, jc * SB:(jc + 1) * SB],
                                             s_ps[:], AF.Exp, scale=1.0 / D)
                        nc.tensor.matmul(
                            rs_ps[:], ones_col,
                            et_sb[:, jc * SB:(jc + 1) * SB],
                            start=(jc == 0), stop=(jc == RCH - 1))

                # rs AllGather + reciprocal, partition-major for per-key scaling
                rs_sb = sp.tile([1, SB], F32)
                nc.vector.tensor_copy(rs_sb[:], rs_ps[:])
                nc.sync.dma_start(out=rs_in[:], in_=rs_sb[:])
                if sim_mode:
                    nc.sync.dma_start(out=rs_out[:, :],
                                      in_=rs_in.to_broadcast([NCORES, SB]))
                else:
                    nc.gpsimd.collective_compute(
                        "AllGather", mybir.AluOpType.bypass, replica_groups=RG,
                        ins=[rs_in.opt()], outs=[rs_out.opt()])
                rs32_sb = sp.tile([RCH, P], F32)
                nc.sync.dma_start(
                    out=rs32_sb[:],
                    in_=rs_out.rearrange("r m -> (r m)").rearrange(
                        "(jc p) -> jc p", p=P))
                rs_p_sb = sp.tile([P, RCH], F32)
                for q in range(P // 32):
                    nc.vector.transpose(rs_p_sb[q * 32:(q + 1) * 32, 0:32],
                                        rs32_sb[0:32, q * 32:(q + 1) * 32])
                recip_sb = sp.tile([P, RCH], F32)
                nc.vector.reciprocal(recip_sb[:], rs_p_sb[:])

                # out_c[m, n] = sum_j ET[j, m] * (1/rs[j]) * V[j, n]
                out_ps = [psum.tile([P, 512], F32, tag="ps", name=f"ops{i}")
                          for i in range(DC // 2 * NH)]
                for r in range(NCORES):
                    vv = v_ag_out[r]
                    for jj in range(JCL):
                        jc = r * JCL + jj
                        v_t = vstp.tile([P, D], BF16, tag="v", name=f"v{jc}")
                        nc.sync.dma_start(out=v_t[:], in_=vv[jj])
                        nc.vector.tensor_scalar_mul(
                            et_sb[:, jc * SB:(jc + 1) * SB],
                            et_sb[:, jc * SB:(jc + 1) * SB],
                            recip_sb[:, jc:jc + 1])
                        for mc in range(SB // P):
                            for h in range(NH):
                                nc.tensor.matmul(
                                    out_ps[mc * NH + h][:],
                                    et_sb[:, jc * SB + mc * P: jc * SB + (mc + 1) * P],
                                    v_t[:, h * 512:(h + 1) * 512],
                                    start=(jc == 0), stop=(jc == RCH - 1))
                for mc in range(SB // P):
                    for h in range(NH):
                        o_t = op.tile([P, 512], F32, tag="o", name=f"o{mc}{h}")
                        if (mc * NH + h) % 2 == 0:
                            nc.vector.tensor_copy(o_t[:], out_ps[mc * NH + h][:])
                        else:
                            nc.scalar.activation(o_t[:], out_ps[mc * NH + h][:],
                                                 AF.Copy)
                        nc.sync.dma_start(
                            out=out[mc * P:(mc + 1) * P, h * 512:(h + 1) * 512],
                            in_=o_t[:])

    nc.compile()
    return nc


def get_nc():
    if "nc" not in _CACHE:
        _CACHE["nc"] = _build_nc()
    return _CACHE["nc"]


F8 = ml_dtypes.float8_e4m3


def _chunked_cols(a, dt=None):
    """[D, X] -> [128, (D//128) * X] with d-chunk-major columns."""
    x = a.shape[1]
    return np.ascontiguousarray(
        a.reshape(DC, P, x).transpose(1, 0, 2).reshape(P, DC * x).astype(dt or BF))


def prepare_in_maps(inputs):
    query = np.asarray(inputs["query"], dtype=np.float32)
    key = np.asarray(inputs["key"], dtype=np.float32)
    value = np.asarray(inputs["value"], dtype=np.float32)
    qT, kT, vT = query.T, key.T, value.T
    wq = _chunked_cols(np.asarray(inputs["Wq"], dtype=np.float32), F8)
    wk = _chunked_cols(np.asarray(inputs["Wk"], dtype=np.float32), F8)
    wv = _chunked_cols(np.asarray(inputs["Wv"], dtype=np.float32))
    bq = np.ascontiguousarray(
        np.asarray(inputs["bq"], dtype=np.float32).reshape(DC, P).T)
    bk = np.ascontiguousarray(
        np.asarray(inputs["bk"], dtype=np.float32).reshape(DC, P).T)
    bv = np.ascontiguousarray(
        np.asarray(inputs["bv"], dtype=np.float32).reshape(1, D).astype(BF))
    in_maps = []
    for c in range(NCORES):
        sl = slice(c * SB, (c + 1) * SB)
        in_maps.append({
            "qt": _chunked_cols(np.ascontiguousarray(qT[:, sl]), F8),
            "kt": _chunked_cols(np.ascontiguousarray(kT[:, sl]), F8),
            "vt": _chunked_cols(np.ascontiguousarray(vT[:, sl])),
            "wq": wq, "wk": wk, "wv": wv,
            "bq": bq, "bk": bk, "bv": bv,
            "onesc": np.ones((P, 1), dtype=BF),
            "onesr": np.ones((1, P), dtype=BF),
        })
    return in_maps


def kernel(**inputs):
    from concourse.bass_utils import run_bass_kernel_spmd

    nc = get_nc()
    in_maps = prepare_in_maps(inputs)
    res = run_bass_kernel_spmd(nc, in_maps, core_ids=list(range(NCORES)))
    return np.ascontiguousarray(
        np.concatenate([r["out"] for r in res.results], axis=0))


if __name__ == "__main__":
    rng = np.random.default_rng(0)
    ins = {
        "query": rng.standard_normal((S, D), dtype=np.float32),
        "key": rng.standard_normal((S, D), dtype=np.float32),
        "value": rng.standard_normal((S, D), dtype=np.float32),
        "Wq": rng.standard_normal((D, D), dtype=np.float32) * 0.05,
        "bq": rng.standard_normal((D,), dtype=np.float32) * 0.05,
        "Wk": rng.standard_normal((D, D), dtype=np.float32) * 0.05,
        "bk": rng.standard_normal((D,), dtype=np.float32) * 0.05,
        "Wv": rng.standard_normal((D, D), dtype=np.float32) * 0.05,
        "bv": rng.standard_normal((D,), dtype=np.float32) * 0.05,
    }
    got = kernel(**ins)
    print("kernel output", got.shape, got.dtype)


# revision 25
# speedup vs baseline: 1.2868x; 1.1053x over previous
"""Trainium2 Bass kernel for the quirky-softmax attention head.

Math (reference):
    Q = query @ Wq + bq ; K = key @ Wk + bk ; V = value @ Wv + bv     [S, D]
    e = exp(Q K^T / D)                                               [S, S]
    weights[i, j] = e[i, j] / rs[j],  rs[j] = sum_k e[j, k]          (column-indexed norm)
    out = weights @ V                                                [S, D]

Strategy (8 NeuronCores, sequence-parallel, single SPMD NEFF):
  * Host pre-transposes query/key/value to [D, S] and hands core c its
    512-column slice, plus the full (reshaped) weights.
  * Core c computes QT_c = Wq^T-form projection of its queries (kept in SBUF),
    and its own 512-row blocks of K^T and V; each block is AllGather'd
    (2 MB/rank, bf16) as soon as it is produced -- AG(K^T) hides behind the
    V/Q projections, AG(V) behind the whole scores phase.
  * Scores are computed directly transposed, ET[j, m] = exp((Q_c K^T)^T / D),
    so the output matmul needs no on-chip transpose. Row sums come from a
    ones-vector matmul accumulated across all key chunks; the per-core [512]
    sums are AllGather'd (2 KB/rank) into the full rs[4096].
  * out_c = (ET * (1/rs)[j]) ^T-contraction @ V, accumulated over all 32 key
    chunks in 8 PSUM banks.
  * Q/K projections and the scores matmul run in fp8e4m3 with
    perf_mode=DoubleRow (256-wide contraction per matmul, ~2x PE rate);
    exp(scores/1024) crushes the fp8 score error ~1000x. The V projection
    and the output matmul stay bf16 (their errors reach the output
    directly). fp32 PSUM accumulation everywhere; measured rel err vs the
    fp32 reference is ~3.5e-3. PSUM pool = 8 banks shared by tag.
"""

import numpy as np
import ml_dtypes

BF = ml_dtypes.bfloat16

S = 4096
D = 1024
NCORES = 8
P = 128
SB = S // NCORES          # 512 queries (and keys) owned per core
DC = D // P               # 8 contraction chunks over D
JCL = SB // P             # 4 local 128-key chunks per core block
RCH = S // P              # 32 global 128-key chunks
NH = D // 512             # 2 halves of the output feature dim

_CACHE = {}


def _build_nc(sim_mode=False):
    import concourse.tile as tile
    from concourse import bacc, mybir

    F32 = mybir.dt.float32
    BF16 = mybir.dt.bfloat16
    FP8 = mybir.dt.float8e4
    AF = mybir.ActivationFunctionType
    RG = [list(range(NCORES))]

    nc = bacc.Bacc("TRN2", target_bir_lowering=False, debug=False,
                   num_devices=NCORES)

    qt = nc.dram_tensor("qt", [P, DC * SB], FP8, kind="ExternalInput").ap()
    kt = nc.dram_tensor("kt", [P, DC * SB], FP8, kind="ExternalInput").ap()
    vt = nc.dram_tensor("vt", [P, DC * SB], BF16, kind="ExternalInput").ap()
    wq = nc.dram_tensor("wq", [P, DC * D], FP8, kind="ExternalInput").ap()
    wk = nc.dram_tensor("wk", [P, DC * D], FP8, kind="ExternalInput").ap()
    wv = nc.dram_tensor("wv", [P, DC * D], BF16, kind="ExternalInput").ap()
    bq = nc.dram_tensor("bq", [P, DC], F32, kind="ExternalInput").ap()
    bk = nc.dram_tensor("bk", [P, DC], F32, kind="ExternalInput").ap()
    bv = nc.dram_tensor("bv", [1, D], BF16, kind="ExternalInput").ap()
    onesc = nc.dram_tensor("onesc", [P, 1], BF16, kind="ExternalInput").ap()
    onesr = nc.dram_tensor("onesr", [1, P], BF16, kind="ExternalInput").ap()
    out = nc.dram_tensor("out", [SB, D], F32, kind="ExternalOutput").ap()

    BLK = DC * P * SB     # 524288 elements in one core's K^T (or V) block

    with tile.TileContext(nc) as tc:
        with (
            tc.tile_pool(name="dram", bufs=1, space="DRAM") as dram,
            tc.tile_pool(name="consts", bufs=1) as consts,
            tc.tile_pool(name="qtp", bufs=1) as qtp,
            tc.tile_pool(name="etp", bufs=1) as etp,
            tc.tile_pool(name="psum", bufs=8, space="PSUM") as psum,
        ):
            kt_ag_in = dram.tile([DC, P, SB], FP8)
            kt_ag_out = dram.tile([NCORES, DC, P, SB], FP8,
                                  addr_space="Local" if sim_mode else "Shared")
            v_ag_in = dram.tile([JCL, P, D], BF16)
            v_ag_out = dram.tile([NCORES, JCL, P, D], BF16,
                                 addr_space="Local" if sim_mode else "Shared")
            rs_in = dram.tile([1, SB], F32)
            rs_out = dram.tile([NCORES, SB], F32,
                               addr_space="Local" if sim_mode else "Shared")


            qt_sb = qtp.tile([P, DC * SB], FP8)       # QT_c resident, fp8
            et_sb = etp.tile([P, RCH * SB], BF16)     # ET resident    (8 MB)

            # ---------------- phase 1: projections + AllGather(KT, V) -------
            with (
                tc.tile_pool(name="inp", bufs=1) as inp,
                tc.tile_pool(name="wp", bufs=4) as wp,
                tc.tile_pool(name="pop", bufs=6) as pop,
            ):
                kt_in = inp.tile([P, DC * SB], FP8)
                vt_in = inp.tile([P, DC * SB], BF16)
                qt_in = inp.tile([P, DC * SB], FP8)

                # KT_c[dout, j'] = sum_d Wk[d, dout] keyT[d, j'] + bk[dout]
                kt_ps = [psum.tile([P, SB], F32, tag="ps", name=f"ktps{m}")
                         for m in range(DC)]
                for c2 in range(DC // 2):
                    kt_w2 = wp.tile([P, 2, D], FP8, tag="w", name=f"wk{c2}")
                    nc.sync.dma_start(
                        out=kt_in[:, 2 * c2 * SB:(2 * c2 + 2) * SB],
                        in_=kt[:, 2 * c2 * SB:(2 * c2 + 2) * SB])
                    nc.sync.dma_start(
                        out=kt_w2.rearrange("p a b -> p (a b)"),
                        in_=wk[:, 2 * c2 * D:(2 * c2 + 2) * D])
                    kt_in3 = kt_in.rearrange("p (dc j) -> p dc j", dc=DC)
                    for mc in range(DC):
                        nc.tensor.matmul(
                            kt_ps[mc][:],
                            kt_w2[:, :, mc * P:(mc + 1) * P],
                            kt_in3[:, 2 * c2:2 * c2 + 2, :],
                            start=(c2 == 0), stop=(c2 == DC // 2 - 1),
                            perf_mode=mybir.MatmulPerfMode.DoubleRow)
                wv_t0 = wp.tile([P, D], BF16, tag="w", name="wv_pre")
                nc.sync.dma_start(out=vt_in[:, 0:SB], in_=vt[:, 0:SB])
                nc.sync.dma_start(out=wv_t0[:], in_=wv[:, 0:D])

                bq_sb = consts.tile([P, DC], F32)
                nc.sync.dma_start(out=bq_sb[:], in_=bq)
                bk_sb = consts.tile([P, DC], F32)
                nc.sync.dma_start(out=bk_sb[:], in_=bk)
                bv_sb = consts.tile([1, D], BF16)
                nc.sync.dma_start(out=bv_sb[:], in_=bv)
                ones_col = consts.tile([P, 1], BF16)
                nc.sync.dma_start(out=ones_col[:], in_=onesc)
                ones_row = consts.tile([1, P], BF16)
                nc.sync.dma_start(out=ones_row[:], in_=onesr)
                for mp in range(DC // 2):
                    kt_o = pop.tile([P, 2, SB], FP8, tag="po8", name=f"kto{mp}")
                    for u in range(2):
                        mc = 2 * mp + u
                        nc.scalar.activation(kt_o[:, u], kt_ps[mc][:], AF.Identity,
                                             bias=bk_sb[:, mc:mc + 1])
                    nc.sync.dma_start(
                        out=kt_ag_in[2 * mp:2 * mp + 2].rearrange("a p j -> p a j"),
                        in_=kt_o[:])

                if sim_mode:
                    for r in range(NCORES):
                        nc.sync.dma_start(out=kt_ag_out[r, :, :, 0:64],
                                          in_=kt_ag_in[:, :, 0:64])
                else:
                    nc.gpsimd.collective_compute(
                        "AllGather", mybir.AluOpType.bypass, replica_groups=RG,
                        ins=[kt_ag_in.opt()], outs=[kt_ag_out.opt()])

                # V_c[j', n] = sum_d valueT[d, j'] Wv[d, n] + bv[n]
                v_ps = [psum.tile([P, 512], F32, tag="ps", name=f"vps{i}")
                        for i in range(JCL * NH)]
                for dc in range(DC):
                    if dc == 0:
                        wv_t = wv_t0
                    else:
                        nc.sync.dma_start(out=vt_in[:, dc * SB:(dc + 1) * SB],
                                          in_=vt[:, dc * SB:(dc + 1) * SB])
                        wv_t = wp.tile([P, D], BF16, tag="w", name=f"wv{dc}")
                        nc.sync.dma_start(out=wv_t[:], in_=wv[:, dc * D:(dc + 1) * D])
                    for jc in range(JCL):
                        for h in range(NH):
                            nc.tensor.matmul(
                                v_ps[jc * NH + h][:],
                                vt_in[:, dc * SB + jc * P: dc * SB + (jc + 1) * P],
                                wv_t[:, h * 512:(h + 1) * 512],
                                start=(dc == 0), stop=False)
                for jc in range(JCL):
                    v_o = pop.tile([P, D], BF16, tag="po", name=f"vo{jc}")
                    for h in range(NH):
                        nc.tensor.matmul(
                            v_ps[jc * NH + h][:],
                            ones_row,
                            bv_sb[:, h * 512:(h + 1) * 512],
                            start=False, stop=True)
                        nc.scalar.activation(v_o[:, h * 512:(h + 1) * 512],
                                             v_ps[jc * NH + h][:], AF.Copy)
                    nc.sync.dma_start(out=v_ag_in[jc], in_=v_o[:])

                if sim_mode:
                    for r in range(NCORES):
                        nc.sync.dma_start(out=v_ag_out[r, :, :, 0:128],
                                          in_=v_ag_in[:, :, 0:128])
                else:
                    nc.gpsimd.collective_compute(
                        "AllGather", mybir.AluOpType.bypass, replica_groups=RG,
                        ins=[v_ag_in.opt()], outs=[v_ag_out.opt()])

                # QT_c[dout, m] = sum_d Wq[d, dout] queryT[d, m] + bq[dout]
                q_ps = [psum.tile([P, SB], F32, tag="ps", name=f"qps{m}")
                        for m in range(DC)]
                for c2 in range(DC // 2):
                    qt_w2 = wp.tile([P, 2, D], FP8, tag="w", name=f"wq{c2}")
                    nc.sync.dma_start(
                        out=qt_in[:, 2 * c2 * SB:(2 * c2 + 2) * SB],
                        in_=qt[:, 2 * c2 * SB:(2 * c2 + 2) * SB])
                    nc.sync.dma_start(
                        out=qt_w2.rearrange("p a b -> p (a b)"),
                        in_=wq[:, 2 * c2 * D:(2 * c2 + 2) * D])
                    qt_in3 = qt_in.rearrange("p (dc m) -> p dc m", dc=DC)
                    for mc in range(DC):
                        nc.tensor.matmul(
                            q_ps[mc][:],
                            qt_w2[:, :, mc * P:(mc + 1) * P],
                            qt_in3[:, 2 * c2:2 * c2 + 2, :],
                            start=(c2 == 0), stop=(c2 == DC // 2 - 1),
                            perf_mode=mybir.MatmulPerfMode.DoubleRow)
                for mc in range(DC):
                    nc.scalar.activation(qt_sb[:, mc * SB:(mc + 1) * SB],
                                         q_ps[mc][:], AF.Identity,
                                         bias=bq_sb[:, mc:mc + 1])

            # ---------------- phases 2+3: scores/exp/rowsums, then output ---
            with (
                tc.tile_pool(name="ktb", bufs=6) as ktbp,
                tc.tile_pool(name="vst", bufs=8) as vstp,
                tc.tile_pool(name="op", bufs=4) as op,
                tc.tile_pool(name="small", bufs=1) as sp,
            ):
                # scores^T per 128-key chunk: ET[j, m] = exp(scores[m, j]/D)
                rs_ps = psum.tile([1, SB], F32, tag="ps")
                for r in range(NCORES):
                    ktb = ktbp.tile([P, DC * SB], FP8, tag="ktb", name=f"ktb{r}")
                    for c2 in range(DC // 2):
                        nc.sync.dma_start(
                            out=ktb[:, 2 * c2 * SB:(2 * c2 + 2) * SB].rearrange(
                                "p (a j) -> p a j", a=2),
                            in_=kt_ag_out[r, 2 * c2:2 * c2 + 2].rearrange(
                                "a p j -> p a j"))
                    for jj in range(JCL):
                        jc = r * JCL + jj
                        s_ps = psum.tile([P, SB], F32, tag="ps", name=f"sps{jc}")
                        ktb3 = ktb.rearrange("p (dc j) -> p dc j", dc=DC)
                        qt3 = qt_sb.rearrange("p (dc m) -> p dc m", dc=DC)
                        for c2 in range(DC // 2):
                            nc.tensor.matmul(
                                s_ps[:],
                                ktb3[:, 2 * c2:2 * c2 + 2, jj * P:(jj + 1) * P],
                                qt3[:, 2 * c2:2 * c2 + 2, :],
                                start=(c2 == 0), stop=(c2 == DC // 2 - 1),
                                perf_mode=mybir.MatmulPerfMode.DoubleRow)
                        nc.scalar.activation(et_sb[:, jc * SB:(jc + 1) * SB],
                                             s_ps[:], AF.Exp, scale=1.0 / D)
                        nc.tensor.matmul(
                            rs_ps[:], ones_col,
                            et_sb[:, jc * SB:(jc + 1) * SB],
                            start=(jc == 0), stop=(jc == RCH - 1))

                # rs AllGather + reciprocal, partition-major for per-key scaling
                rs_sb = sp.tile([1, SB], F32)
                nc.vector.tensor_copy(rs_sb[:], rs_ps[:])
                nc.sync.dma_start(out=rs_in[:], in_=rs_sb[:])
                if sim_mode:
                    nc.sync.dma_start(out=rs_out[:, :],
                                      in_=rs_in.to_broadcast([NCORES, SB]))
                else:
                    nc.gpsimd.collective_compute(
                        "AllGather", mybir.AluOpType.bypass, replica_groups=RG,
                        ins=[rs_in.opt()], outs=[rs_out.opt()])
                rs32_sb = sp.tile([RCH, P], F32)
                nc.sync.dma_start(
                    out=rs32_sb[:],
                    in_=rs_out.rearrange("r m -> (r m)").rearrange(
                        "(jc p) -> jc p", p=P))
                rs_p_sb = sp.tile([P, RCH], F32)
                for q in range(P // 32):
                    nc.vector.transpose(rs_p_sb[q * 32:(q + 1) * 32, 0:32],
                                        rs32_sb[0:32, q * 32:(q + 1) * 32])
                recip_sb = sp.tile([P, RCH], F32)
                nc.vector.reciprocal(recip_sb[:], rs_p_sb[:])

                # out_c[m, n] = sum_j ET[j, m] * (1/rs[j]) * V[j, n]
                out_ps = [psum.tile([P, 512], F32, tag="ps", name=f"ops{i}")
                          for i in range(DC // 2 * NH)]
                for r in range(NCORES):
                    vv = v_ag_out[r]
                    for jp in range(JCL // 2):
                        v_t = vstp.tile([P, 2, D], BF16, tag="v",
                                        name=f"v{r}{jp}")
                        nc.sync.dma_start(
                            out=v_t[:],
                            in_=vv[2 * jp:2 * jp + 2].rearrange("a p n -> p a n"))
                        for u in range(2):
                            jc = r * JCL + 2 * jp + u
                            nc.vector.tensor_scalar_mul(
                                et_sb[:, jc * SB:(jc + 1) * SB],
                                et_sb[:, jc * SB:(jc + 1) * SB],
                                recip_sb[:, jc:jc + 1])
                            for mc in range(SB // P):
                                for h in range(NH):
                                    nc.tensor.matmul(
                                        out_ps[mc * NH + h][:],
                                        et_sb[:, jc * SB + mc * P: jc * SB + (mc + 1) * P],
                                        v_t[:, u, h * 512:(h + 1) * 512],
                                        start=(jc == 0), stop=(jc == RCH - 1))
                for mc in range(SB // P):
                    o_t = op.tile([P, D], F32, tag="o", name=f"o{mc}")
                    for h in range(NH):
                        if h == 0:
                            nc.vector.tensor_copy(o_t[:, 0:512],
                                                  out_ps[mc * NH][:])
                        else:
                            nc.scalar.activation(o_t[:, 512:D],
                                                 out_ps[mc * NH + 1][:], AF.Copy)
                    nc.sync.dma_start(out=out[mc * P:(mc + 1) * P, :], in_=o_t[:])

    nc.compile()
    return nc


def get_nc():
    if "nc" not in _CACHE:
        _CACHE["nc"] = _build_nc()
    return _CACHE["nc"]


F8 = ml_dtypes.float8_e4m3


def _chunked_cols(a, dt=None):
    """[D, X] -> [128, (D//128) * X] with d-chunk-major columns."""
    x = a.shape[1]
    return np.ascontiguousarray(
        a.reshape(DC, P, x).transpose(1, 0, 2).reshape(P, DC * x).astype(dt or BF))


def prepare_in_maps(inputs):
    query = np.asarray(inputs["query"], dtype=np.float32)
    key = np.asarray(inputs["key"], dtype=np.float32)
    value = np.asarray(inputs["value"], dtype=np.float32)
    qT, kT, vT = query.T, key.T, value.T
    wq = _chunked_cols(np.asarray(inputs["Wq"], dtype=np.float32), F8)
    wk = _chunked_cols(np.asarray(inputs["Wk"], dtype=np.float32), F8)
    wv = _chunked_cols(np.asarray(inputs["Wv"], dtype=np.float32))
    bq = np.ascontiguousarray(
        np.asarray(inputs["bq"], dtype=np.float32).reshape(DC, P).T)
    bk = np.ascontiguousarray(
        np.asarray(inputs["bk"], dtype=np.float32).reshape(DC, P).T)
    bv = np.ascontiguousarray(
        np.asarray(inputs["bv"], dtype=np.float32).reshape(1, D).astype(BF))
    in_maps = []
    for c in range(NCORES):
        sl = slice(c * SB, (c + 1) * SB)
        in_maps.append({
            "qt": _chunked_cols(np.ascontiguousarray(qT[:, sl]), F8),
            "kt": _chunked_cols(np.ascontiguousarray(kT[:, sl]), F8),
            "vt": _chunked_cols(np.ascontiguousarray(vT[:, sl])),
            "wq": wq, "wk": wk, "wv": wv,
            "bq": bq, "bk": bk, "bv": bv,
            "onesc": np.ones((P, 1), dtype=BF),
            "onesr": np.ones((1, P), dtype=BF),
        })
    return in_maps


def kernel(**inputs):
    from concourse.bass_utils import run_bass_kernel_spmd

    nc = get_nc()
    in_maps = prepare_in_maps(inputs)
    res = run_bass_kernel_spmd(nc, in_maps, core_ids=list(range(NCORES)))
    return np.ascontiguousarray(
        np.concatenate([r["out"] for r in res.results], axis=0))


if __name__ == "__main__":
    rng = np.random.default_rng(0)
    ins = {
        "query": rng.standard_normal((S, D), dtype=np.float32),
        "key": rng.standard_normal((S, D), dtype=np.float32),
        "value": rng.standard_normal((S, D), dtype=np.float32),
        "Wq": rng.standard_normal((D, D), dtype=np.float32) * 0.05,
        "bq": rng.standard_normal((D,), dtype=np.float32) * 0.05,
        "Wk": rng.standard_normal((D, D), dtype=np.float32) * 0.05,
        "bk": rng.standard_normal((D,), dtype=np.float32) * 0.05,
        "Wv": rng.standard_normal((D, D), dtype=np.float32) * 0.05,
        "bv": rng.standard_normal((D,), dtype=np.float32) * 0.05,
    }
    got = kernel(**ins)
    print("kernel output", got.shape, got.dtype)
